# revision 3
# baseline (speedup 1.0000x reference)
"""Lie-series expansion kernel for Trainium2 (8 NeuronCores, data-parallel).

result = x + sum_{i=1..order} z_i,  z_i = (1/i) * sum_g diag(theta_g) z_{i-1} A_g

Per step the G=8 generator contraction fuses into ONE [B,4096]x[4096,512]
matmul: stack W_(g,f) = (theta_g/i) * z_{i-1} along the contraction dim.
Data-parallel over batch: each core owns B/8=512 rows, keeps z TRANSPOSED
([feature_partitions, batch_free]) so the theta scaling is a DVE
elementwise op and algebra A[g,f,h] is the stationary operand in natural
layout. Everything lives in SBUF across all steps; float32r matmuls run
at 1 cycle/row (4x over plain fp32).
"""

import numpy as np

import concourse.bass as bass
import concourse.bacc as bacc
import concourse.mybir as mybir
from concourse import tile
from concourse.bass_utils import run_bass_kernel_spmd

G, B, F = 8, 4096, 512
NCORES = 8
BLOC = B // NCORES          # 512 batch rows per core
P = 128                     # partitions
FT = F // P                 # 4 feature tiles
NK = G * FT                 # 32 contraction k-tiles per step
DT = mybir.dt.float32
DTR = mybir.dt.float32r
MULT = mybir.AluOpType.mult

_cache = {}


def _build(order: int):
    if order in _cache:
        return _cache[order]

    nc = bacc.Bacc("TRN2", target_bir_lowering=False, debug=False,
                   num_devices=NCORES)

    A_d = nc.dram_tensor("A", [P, NK * F], DTR, kind="ExternalInput")
    th_d = nc.dram_tensor("th", [P, G * BLOC], DT, kind="ExternalInput")
    xT_d = nc.dram_tensor("xT", [F, BLOC], DT, kind="ExternalInput")
    out_d = nc.dram_tensor("outT", [F, BLOC], DT, kind="ExternalOutput")

    with tile.TileContext(nc) as tc:
        with (
            tc.tile_pool(name="const", bufs=1) as cpool,
            tc.tile_pool(name="z", bufs=2) as zpool,
            tc.tile_pool(name="w", bufs=10) as wpool,
            tc.tile_pool(name="psum", bufs=2, space=bass.MemorySpace.PSUM) as ppool,
        ):
            th = cpool.tile([P, G * BLOC], DT, tag="th", name="th")
            nc.sync.dma_start(th[:], th_d[:])

            res = []
            zT = []
            for ft in range(FT):
                zt = zpool.tile([P, BLOC], DT, tag=f"z{ft}", name=f"z{ft}_init")
                nc.sync.dma_start(zt[:], xT_d[ft * P:(ft + 1) * P, :])
                rt = cpool.tile([P, BLOC], DT, tag=f"res{ft}", name=f"res{ft}")
                nc.scalar.copy(rt[:], zt[:])
                res.append(rt)
                zT.append(zt)

            # A k-tiles as separate tiles for fine-grained DMA/compute overlap,
            # loaded in the order step 1 consumes them.
            A_t = []
            for k in range(NK):
                at = cpool.tile([P, F], DTR, tag=f"A{k}", name=f"A{k}")
                nc.sync.dma_start(at[:], A_d[:, k * F:(k + 1) * F])
                A_t.append(at)

            for i in range(1, order + 1):
                s = 1.0 / i
                ps = [ppool.tile([P, BLOC], DT, tag=f"ps{ho}", name=f"ps{ho}_{i}") for ho in range(FT)]
                for fi in range(FT):
                    for g in range(G):
                        k = g * FT + fi
                        w = wpool.tile([P, BLOC], DTR, tag="w", name=f"w_{i}_{k}")
                        # w = (theta_g * 1/i) * zT[fi]
                        nc.vector.scalar_tensor_tensor(
                            w[:], th[:, g * BLOC:(g + 1) * BLOC], s, zT[fi][:],
                            MULT, MULT)
                        first = (fi == 0 and g == 0)
                        last = (fi == FT - 1 and g == G - 1)
                        for ho in range(FT):
                            nc.tensor.matmul(
                                ps[ho][:],
                                A_t[k][:, ho * P:(ho + 1) * P],
                                w[:],
                                start=first, stop=last)
                zT_new = []
                for ho in range(FT):
                    zt = zpool.tile([P, BLOC], DT, tag=f"z{ho}", name=f"z{ho}_{i}")
                    nc.scalar.copy(zt[:], ps[ho][:])
                    nc.vector.tensor_add(res[ho][:], res[ho][:], zt[:])
                    zT_new.append(zt)
                zT = zT_new

            for ft in range(FT):
                nc.sync.dma_start(out_d[ft * P:(ft + 1) * P, :], res[ft][:])

    nc.compile()
    _cache[order] = nc
    return nc


def _in_maps(theta, x, algebra):
    theta = np.ascontiguousarray(theta, dtype=np.float32)
    x = np.ascontiguousarray(x, dtype=np.float32)
    algebra = np.ascontiguousarray(algebra, dtype=np.float32)
    # A[g, f, h] -> A_host[p, (g*FT+fi)*F + h] with f = fi*128 + p
    A_host = np.ascontiguousarray(
        algebra.reshape(G, FT, P, F).transpose(2, 0, 1, 3).reshape(P, NK * F))
    maps = []
    for c in range(NCORES):
        th_loc = theta[:, c * BLOC:(c + 1) * BLOC]          # [G, BLOC]
        th_b = np.ascontiguousarray(
            np.broadcast_to(th_loc[None], (P, G, BLOC)).reshape(P, G * BLOC))
        xT = np.ascontiguousarray(x[c * BLOC:(c + 1) * BLOC, :].T)
        maps.append({"A": A_host, "th": th_b, "xT": xT})
    return maps


def _run(theta, x, algebra, order, **kw):
    nc = _build(int(order))
    res = run_bass_kernel_spmd(nc, _in_maps(theta, x, algebra),
                               list(range(NCORES)), **kw)
    out = np.empty((B, F), dtype=np.float32)
    for c in range(NCORES):
        out[c * BLOC:(c + 1) * BLOC, :] = res.results[c]["outT"].T
    return out, res


def kernel(theta, x, algebra, order):
    out, _ = _run(theta, x, algebra, order)
    return out


# revision 5
# speedup vs baseline: 1.1844x; 1.1844x over previous
"""Lie-series expansion kernel for Trainium2 (8 NeuronCores, data-parallel).

result = x + sum_{i=1..order} z_i,  z_i = (1/i) * sum_g diag(theta_g) z_{i-1} A_g

Per step the G=8 generator contraction fuses into ONE [B,4096]x[4096,512]
matmul: stack W_(g,f) = (theta_g/i) * z_{i-1} along the contraction dim.
Data-parallel over batch: each core owns B/8=512 rows, keeps z TRANSPOSED
([feature_partitions, batch_free]) so the theta scaling is a DVE
elementwise op and algebra A[g,f,h] is the stationary operand in natural
layout. Everything lives in SBUF across all steps; float32r matmuls run
at 1 cycle/row (4x over plain fp32).
"""

import numpy as np

import concourse.bass as bass
import concourse.bacc as bacc
import concourse.mybir as mybir
from concourse import tile
from concourse.bass_utils import run_bass_kernel_spmd

G, B, F = 8, 4096, 512
NCORES = 8
BLOC = B // NCORES          # 512 batch rows per core
P = 128                     # partitions
FT = F // P                 # 4 feature tiles
NK = G * FT                 # 32 contraction k-tiles per step
DT = mybir.dt.float32
DTR = mybir.dt.float32r
MULT = mybir.AluOpType.mult

_cache = {}


def _build(order: int):
    if order in _cache:
        return _cache[order]

    nc = bacc.Bacc("TRN2", target_bir_lowering=False, debug=False,
                   num_devices=NCORES)

    A_d = nc.dram_tensor("A", [P, NK * F], DTR, kind="ExternalInput")
    th_d = nc.dram_tensor("th", [P, G * BLOC], DT, kind="ExternalInput")
    xT_d = nc.dram_tensor("xT", [F, BLOC], DT, kind="ExternalInput")
    out_d = nc.dram_tensor("outT", [F, BLOC], DT, kind="ExternalOutput")

    # Consumption order of contraction k-tiles: fi-major (fi outer, g inner).
    korder = [g * FT + fi for fi in range(FT) for g in range(G)]

    with tile.TileContext(nc) as tc:
        with (
            tc.tile_pool(name="const", bufs=1) as cpool,
            tc.tile_pool(name="z", bufs=2) as zpool,
            tc.tile_pool(name="w", bufs=2) as wpool,
            tc.tile_pool(name="psum", bufs=2, space=bass.MemorySpace.PSUM) as ppool,
        ):
            # theta chunks + x^T on the gpsimd DMA queue, A k-tiles on the
            # sync queue — both in consumption order so step 1 chases DMA.
            zT = []
            for ft in range(FT):
                zt = zpool.tile([P, BLOC], DT, tag=f"z{ft}", name=f"z{ft}_init")
                nc.gpsimd.dma_start(zt[:], xT_d[ft * P:(ft + 1) * P, :])
                zT.append(zt)
            th = []
            for g in range(G):
                tg = cpool.tile([P, BLOC], DT, tag=f"th{g}", name=f"th{g}")
                nc.gpsimd.dma_start(tg[:], th_d[:, g * BLOC:(g + 1) * BLOC])
                th.append(tg)
            res = []
            for ft in range(FT):
                rt = cpool.tile([P, BLOC], DT, tag=f"res{ft}", name=f"res{ft}")
                nc.scalar.copy(rt[:], zT[ft][:])
                res.append(rt)

            A_t = [None] * NK
            for k in korder:
                at = cpool.tile([P, F], DTR, tag=f"A{k}", name=f"A{k}")
                nc.sync.dma_start(at[:], A_d[:, k * F:(k + 1) * F])
                A_t[k] = at

            def w_build(i, k, src):
                g = k // FT
                w = wpool.tile([P, BLOC], DTR, tag="w", bufs=40,
                               name=f"w_{i}_{k}")
                # w = (theta_g / i) * z_{i-1}
                nc.vector.scalar_tensor_tensor(
                    w[:], th[g][:, :], 1.0 / i, src[:], MULT, MULT)
                return w

            def drain(i, ps_ho, ho, Wn):
                """Consume step i's completed psum bank `ho`: either fold into
                res (last step) or copy to SBUF, accumulate, and build step
                i+1's W tiles for fi=ho."""
                if i == order:
                    nc.vector.scalar_tensor_tensor(
                        res[ho][:], ps_ho[:], 1.0, res[ho][:],
                        MULT, mybir.AluOpType.add)
                    nc.sync.dma_start(out_d[ho * P:(ho + 1) * P, :],
                                      res[ho][:])
                else:
                    zt = zpool.tile([P, BLOC], DT, tag=f"z{ho}",
                                    name=f"z{ho}_{i}")
                    nc.scalar.copy(zt[:], ps_ho[:])
                    nc.vector.tensor_add(res[ho][:], res[ho][:], zt[:])
                    for g in range(G):
                        Wn[g * FT + ho] = w_build(i + 1, g * FT + ho, zt)

            # ---- step 1: fi-outer, W built inline from x^T (chases DMA) ----
            W = [None] * NK
            ps = [ppool.tile([P, BLOC], DT, tag=f"ps{ho}", name=f"ps{ho}_1")
                  for ho in range(FT)]
            for n, k in enumerate(korder):
                W[k] = w_build(1, k, zT[k % FT])
                for ho in range(FT):
                    nc.tensor.matmul(
                        ps[ho][:], A_t[k][:, ho * P:(ho + 1) * P], W[k][:],
                        start=(n == 0), stop=(n == NK - 1))
            Wn = [None] * NK
            for ho in range(FT):
                drain(1, ps[ho], ho, Wn)
            W = Wn

            # ---- steps 2..order: ho-outer so psum banks complete early and
            # step i+1's W tiles pre-build during step i (no boundary bubble)
            for i in range(2, order + 1):
                Wn = [None] * NK
                psn = [ppool.tile([P, BLOC], DT, tag=f"ps{ho}",
                                  name=f"ps{ho}_{i}") for ho in range(FT)]
                for ho in range(FT):
                    for n, k in enumerate(korder):
                        nc.tensor.matmul(
                            psn[ho][:], A_t[k][:, ho * P:(ho + 1) * P], W[k][:],
                            start=(n == 0), stop=(n == NK - 1))
                    drain(i, psn[ho], ho, Wn)
                W = Wn

    nc.compile()
    _cache[order] = nc
    return nc


def _in_maps(theta, x, algebra):
    theta = np.ascontiguousarray(theta, dtype=np.float32)
    x = np.ascontiguousarray(x, dtype=np.float32)
    algebra = np.ascontiguousarray(algebra, dtype=np.float32)
    # A[g, f, h] -> A_host[p, (g*FT+fi)*F + h] with f = fi*128 + p
    A_host = np.ascontiguousarray(
        algebra.reshape(G, FT, P, F).transpose(2, 0, 1, 3).reshape(P, NK * F))
    maps = []
    for c in range(NCORES):
        th_loc = theta[:, c * BLOC:(c + 1) * BLOC]          # [G, BLOC]
        th_b = np.ascontiguousarray(
            np.broadcast_to(th_loc[None], (P, G, BLOC)).reshape(P, G * BLOC))
        xT = np.ascontiguousarray(x[c * BLOC:(c + 1) * BLOC, :].T)
        maps.append({"A": A_host, "th": th_b, "xT": xT})
    return maps


def _run(theta, x, algebra, order, **kw):
    nc = _build(int(order))
    res = run_bass_kernel_spmd(nc, _in_maps(theta, x, algebra),
                               list(range(NCORES)), **kw)
    out = np.empty((B, F), dtype=np.float32)
    for c in range(NCORES):
        out[c * BLOC:(c + 1) * BLOC, :] = res.results[c]["outT"].T
    return out, res


def kernel(theta, x, algebra, order):
    out, _ = _run(theta, x, algebra, order)
    return out
